# revision 17
# baseline (speedup 1.0000x reference)
"""Trainium2 Bass kernel for CompressedLinear: out = x @ (w_int8 * scale).T + bias.

Sharding (Megatron column-parallel): weight/scale/bias are split along the
output dim across 8 NeuronCores, x is replicated, per-core outputs are
concatenated on the feature axis.

Mixed-precision fp8 DoubleRow scheme (24 PE instructions per PSUM tile vs 32
for an exact fp16 pass -- 0.75x the PE row streaming, which is the hard
bottleneck at 2.4 GHz / 1 row/cycle):

- Weights are centered (w - 63) and encoded once as e4m3(alpha*(w-63)) with a
  global alpha = 1.0125 that minimizes the quantization rms against the e4m3
  grid (rms 0.83 vs 0.94 at alpha=1).  The DC part (63) is exact via a
  rank-1 correction: out += 63*rowsum(x)[t]*scale[o], folded into PSUM
  eviction at zero extra cost: out = (ps + 63*alpha*rowsum(x)) * (scale/alpha).
- x is split into x_hi = e4m3(x) and x_lo = e4m3(x - x_hi).  For the first
  LO_BLOCKS (16) of the 32 k-blocks, the DoubleRow plane pair carries
  (x_hi[k], x_lo[k]) against a duplicated weight plane -- high x precision.
  For the remaining 16 k-blocks the pair carries (x_hi[k0], x_hi[k1]) against
  (w8[k0], w8[k1]) -- two k-blocks per instruction.
  => 16 + 8 = 24 DoubleRow matmuls per [128,512] PSUM tile.

Error budget (verified bit-faithful against numpy sim on the real inputs):
rel err ~1.77e-2 vs the 2e-2 gate (x-rounding coupling on the k-doubled half
~1.12e-2, fp8 weight quantization ~1.27e-2).
"""

import numpy as np
import ml_dtypes

import concourse.bass as bass
import concourse.mybir as mybir
import concourse.tile as tile
from concourse import bacc
from concourse.bass_utils import run_bass_kernel_spmd

B, S, IN, OUT = 4, 2048, 4096, 11008
N_CORES = 8
TOK = B * S
O_CORE = OUT // N_CORES
P = 128

M_TILE = 256
N_TILE = 512
X_BUFS = 4
PSUM_BUFS = 6
KG = 8          # k-planes per startup weight-DMA chunk
LO_BLOCKS = 16  # k-blocks (of 32) whose DoubleRow pair carries (x_hi, x_lo)
ALPHA = 1.0125  # global fp8 weight-encoding scale
W_BCAST = 1     # 1: lo-group w planes via stride-0 AP; 0: duplicate in DRAM

F8 = ml_dtypes.float8_e4m3  # trn2 float8e4 (e4m3, max 240)


def build_nc(tok=TOK, k_dim=IN, o_core=O_CORE,
             m_tile=M_TILE, n_tile=N_TILE, x_bufs=X_BUFS, psum_bufs=PSUM_BUFS,
             lo_blocks=LO_BLOCKS, w_bcast=W_BCAST):
    assert k_dim % P == 0 and tok % m_tile == 0 and m_tile % P == 0
    ksub = k_dim // P                      # real k-blocks (32)
    assert lo_blocks % 2 == 0 and (ksub - lo_blocks) % 2 == 0
    planes = 2 * lo_blocks + (ksub - lo_blocks)   # x plane count (48)
    w_planes = ksub if w_bcast else planes        # w plane count (32 or 48)
    n_mm = planes // 2                     # DoubleRow matmuls per psum tile (24)
    msub = m_tile // P
    n_mtiles = tok // m_tile
    n_slices = [(n0, min(n_tile, o_core - n0)) for n0 in range(0, o_core, n_tile)]
    f8 = mybir.dt.float8e4

    # w chunk offsets into the per-partition packed layout (see _shard_inputs)
    w_offs = {}
    off = 0
    for n_idx, (n0, nsz) in enumerate(n_slices):
        for kg in range(0, w_planes, KG):
            kgn = min(KG, w_planes - kg)
            w_offs[(n_idx, kg // KG)] = (off, kgn, nsz)
            off += kgn * nsz
    w_total = off

    nc = bacc.Bacc(None, target_bir_lowering=False)
    # xt is pre-tiled host-side: per m-tile, each partition's 48 plane
    # fragments are contiguous (12 KB lines -> full-rate DMA).
    xt = nc.declare_dram_parameter("xt", [n_mtiles * P, planes * m_tile], f8,
                                   False)
    # wt is packed per partition in chunk-consumption order (4 KB lines).
    wt = nc.declare_dram_parameter("wt", [P, w_total], f8, False)
    rs63a = nc.declare_dram_parameter("rs63a", [tok], mybir.dt.float32, False)
    sinva = nc.declare_dram_parameter("sinva", [o_core], mybir.dt.float32, False)
    bias = nc.declare_dram_parameter("bias", [o_core], mybir.dt.float32, False)
    out = nc.declare_dram_parameter("out", [tok, o_core], mybir.dt.float32, True)

    with tile.TileContext(nc) as tc:
        with (
            tc.tile_pool(name="const", bufs=1) as cpool,
            tc.tile_pool(name="xp", bufs=x_bufs) as xpool,
            tc.tile_pool(name="op", bufs=2) as opool,
            tc.tile_pool(name="ps", bufs=psum_bufs, space="PSUM") as pspool,
        ):
            xt_re = xt.rearrange("(m p) f -> m p f", p=P)
            out_re = out.rearrange("(m p) o -> m p o", p=P)
            rs_re = rs63a.rearrange("(m ms p) -> p m ms", ms=msub, p=P)

            # x tiles stream on the sync queue; resident weights load on the
            # scalar queue so they don't serialize behind/ahead of x; the
            # scale/bias broadcasts queue behind the weights so the first
            # x tile gets the bandwidth.
            x_pre = xpool.tile([P, planes, m_tile], f8, tag="x")
            nc.sync.dma_start(out=x_pre[:], in_=xt_re[0])

            w_tiles = {}
            w_queues = [nc.scalar, nc.gpsimd]
            wq = 0
            for n_idx, (n0, nsz) in enumerate(n_slices):
                for kg in range(0, w_planes, KG):
                    kgi = kg // KG
                    off, kgn, _ = w_offs[(n_idx, kgi)]
                    w_t = cpool.tile([P, kgn, nsz], f8, tag=f"w_{n_idx}_{kg}")
                    w_queues[wq % 2].dma_start(out=w_t[:],
                                               in_=wt[:, off:off + kgn * nsz])
                    wq += 1
                    w_tiles[(n_idx, kgi)] = w_t
            sinva_sb = cpool.tile([P, o_core], mybir.dt.float32)
            nc.scalar.dma_start(out=sinva_sb[:],
                                in_=sinva[None, :].to_broadcast((P, o_core)))
            bias_sb = cpool.tile([P, o_core], mybir.dt.float32)
            nc.scalar.dma_start(out=bias_sb[:],
                                in_=bias[None, :].to_broadcast((P, o_core)))
            rs_sb = cpool.tile([P, n_mtiles, msub], mybir.dt.float32)
            nc.scalar.dma_start(out=rs_sb[:], in_=rs_re[:, :, :])

            for mi in range(n_mtiles):
                if mi == 0:
                    x_sb = x_pre
                else:
                    x_sb = xpool.tile([P, planes, m_tile], f8, tag="x")
                    nc.sync.dma_start(out=x_sb[:], in_=xt_re[mi])
                out_sb = opool.tile([P, msub, o_core], mybir.dt.float32, tag="o")
                for ms in range(msub):
                    for n_idx, (n0, nsz) in enumerate(n_slices):
                        ps = pspool.tile([P, n_tile], mybir.dt.float32, tag="ps")
                        psv = ps[:, :nsz]
                        for t in range(n_mm):
                            if w_bcast:
                                wk = t if t < lo_blocks else 2 * t - lo_blocks
                            else:
                                wk = 2 * t
                            w_t = w_tiles[(n_idx, wk // KG)]
                            ko = wk % KG
                            if w_bcast and t < lo_blocks:
                                rhs = w_t[:, ko:ko + 1, :nsz].to_broadcast(
                                    (P, 2, nsz))
                            else:
                                rhs = w_t[:, ko:ko + 2, :nsz]
                            nc.tensor.matmul(
                                psv,
                                lhsT=x_sb[:, 2 * t:2 * t + 2, ms * P:(ms + 1) * P],
                                rhs=rhs,
                                start=(t == 0),
                                stop=(t == n_mm - 1),
                                perf_mode=mybir.MatmulPerfMode.DoubleRow,
                            )
                        # out = (psum + 63*alpha*rowsum(x)[t]) * (scale/alpha)[o]
                        nc.vector.scalar_tensor_tensor(
                            out=out_sb[:, ms, n0:n0 + nsz],
                            in0=psv,
                            scalar=rs_sb[:, mi, ms:ms + 1],
                            op0=mybir.AluOpType.add,
                            op1=mybir.AluOpType.mult,
                            in1=sinva_sb[:, n0:n0 + nsz],
                        )
                        nc.vector.tensor_add(out=out_sb[:, ms, n0:n0 + nsz],
                                             in0=out_sb[:, ms, n0:n0 + nsz],
                                             in1=bias_sb[:, n0:n0 + nsz])
                        nc.sync.dma_start(
                            out=out_re[mi * msub + ms][:, n0:n0 + nsz],
                            in_=out_sb[:, ms, n0:n0 + nsz])
    nc.compile()
    return nc


def _shard_inputs(x2d, w, scale, bias, n_cores=N_CORES, o_core=O_CORE,
                  lo_blocks=LO_BLOCKS, w_bcast=W_BCAST,
                  m_tile=M_TILE, n_tile=N_TILE):
    ksub = IN // P
    x_hi = x2d.astype(F8)                                 # [TOK, IN] e4m3
    e = x2d - x_hi.astype(np.float32)
    x_lo = e.astype(F8)
    rs63a = np.ascontiguousarray(
        63.0 * ALPHA * x2d.sum(axis=1, dtype=np.float64)).astype(np.float32)

    # x plane order: [x_hi[k], x_lo[k]] pairs for k < lo_blocks, then
    # x_hi[lo_blocks:].  Pre-tiled so each m-tile is one contiguous
    # [planes*m_tile] line per partition: [mi, p, plane, t'].
    xhiT = np.ascontiguousarray(x_hi.T).reshape(ksub, P, TOK)
    xloT = np.ascontiguousarray(x_lo.T).reshape(ksub, P, TOK)
    x_planes = []
    for k in range(lo_blocks):
        x_planes += [xhiT[k], xloT[k]]
    for k in range(lo_blocks, ksub):
        x_planes.append(xhiT[k])
    planes = len(x_planes)
    n_mtiles = TOK // m_tile
    arr = np.stack(x_planes, axis=0)                      # [planes, P, TOK]
    arr = arr.reshape(planes, P, n_mtiles, m_tile)
    xt = np.ascontiguousarray(arr.transpose(2, 1, 0, 3)   # [mi, p, plane, t']
                              ).reshape(n_mtiles * P, planes * m_tile)

    common = {"xt": xt, "rs63a": rs63a}
    in_maps = []
    for c in range(n_cores):
        sl = slice(c * o_core, (c + 1) * o_core)
        w_c = w[sl].astype(np.float32) - 63.0             # [o_core, IN]
        w8 = (w_c * ALPHA).astype(F8)                     # encoded fp8 weights
        w8T = np.ascontiguousarray(w8.T).reshape(ksub, P, o_core)
        if w_bcast:
            wp = w8T                                      # [32, P, o_core]
        else:
            w_planes = []
            for k in range(lo_blocks):
                w_planes += [w8T[k], w8T[k]]
            for k in range(lo_blocks, ksub):
                w_planes.append(w8T[k])
            wp = np.stack(w_planes, axis=0)
        # pack per partition in chunk-consumption order: per (n-slice,
        # k-group) a contiguous [kgn*nsz] block (4 KB DMA lines)
        blocks = []
        for n0 in range(0, o_core, n_tile):
            nsz = min(n_tile, o_core - n0)
            for kg in range(0, wp.shape[0], KG):
                kgn = min(KG, wp.shape[0] - kg)
                blk = wp[kg:kg + kgn, :, n0:n0 + nsz]     # [kgn, P, nsz]
                blocks.append(blk.transpose(1, 0, 2).reshape(P, kgn * nsz))
        wt = np.ascontiguousarray(np.concatenate(blocks, axis=1))  # [P, total]
        in_maps.append({
            **common,
            "wt": wt,
            "sinva": np.ascontiguousarray(scale[sl] / ALPHA),
            "bias": np.ascontiguousarray(bias[sl]),
        })
    return in_maps


def _ensure_ntff_hook():
    """Register the axon NTFF profiling hook if the image's antenv lacks it."""
    import sys, types
    try:
        from antenv.axon_hooks import get_axon_ntff_profile_hook  # noqa: F401
        return
    except ImportError:
        pass
    try:
        import antenv
        from trn_agent_boot.trn_boot import _ntff_profile_via_ctypes
        mod = types.ModuleType("antenv.axon_hooks")
        _hook = [_ntff_profile_via_ctypes("/opt/axon/libaxon_pjrt.so")]
        mod.set_axon_ntff_profile_hook = lambda h: _hook.__setitem__(0, h)
        mod.get_axon_ntff_profile_hook = lambda: _hook[0]
        sys.modules["antenv.axon_hooks"] = mod
        antenv.axon_hooks = mod
    except Exception as e:  # profiling is best-effort; execution still works
        print(f"NTFF hook registration failed: {e}")


def run_hw(x2d, w, scale, bias, trace=False, **build_kwargs):
    """Run sharded on 8 cores; returns (full [TOK, OUT] f32 output, exec_time_ns)."""
    if trace:
        _ensure_ntff_hook()
    nc = build_nc(**build_kwargs)
    in_maps = _shard_inputs(x2d, w, scale, bias,
                            lo_blocks=build_kwargs.get("lo_blocks", LO_BLOCKS),
                            w_bcast=build_kwargs.get("w_bcast", W_BCAST),
                            m_tile=build_kwargs.get("m_tile", M_TILE),
                            n_tile=build_kwargs.get("n_tile", N_TILE))
    last_err = None
    for attempt in range(3):
        try:
            res = run_bass_kernel_spmd(nc, in_maps, core_ids=list(range(N_CORES)),
                                       trace=trace)
            out = np.concatenate([res.results[c]["out"] for c in range(N_CORES)],
                                 axis=1)
            return out, res.exec_time_ns
        except Exception as e:  # transient NRT_EXEC_UNIT_UNRECOVERABLE etc.
            last_err = e
            print(f"run attempt {attempt} failed: {type(e).__name__}: {e}")
            try:
                import jax
                import jax.extend.backend as _jb
                jax.clear_caches()
                _jb.clear_backends()
            except Exception as e2:
                print(f"backend reset failed: {e2}")
            import time
            time.sleep(5)
    raise last_err


def kernel(**inputs):
    x = np.asarray(inputs["x"], dtype=np.float32)
    w = np.asarray(inputs["weight_int8"]).astype(np.int32)
    scale = np.asarray(inputs["scale"], dtype=np.float32)
    bias = np.asarray(inputs["bias"], dtype=np.float32)
    out2d, _ = run_hw(x.reshape(TOK, IN), w, scale, bias, trace=False)
    return out2d.reshape(B, S, OUT)


# revision 19
# speedup vs baseline: 1.1944x; 1.1944x over previous
"""Trainium2 Bass kernel for CompressedLinear: out = x @ (w_int8 * scale).T + bias.

Sharding (Megatron column-parallel): weight/scale/bias are split along the
output dim across 8 NeuronCores, x is replicated, per-core outputs are
concatenated on the feature axis.

Mixed-precision fp8 DoubleRow scheme (24 PE instructions per PSUM tile vs 32
for an exact fp16 pass -- 0.75x the PE row streaming, which is the hard
bottleneck at 2.4 GHz / 1 row/cycle):

- Weights are centered (w - 63) and encoded once as e4m3(alpha*(w-63)) with a
  global alpha = 1.0125 that minimizes the quantization rms against the e4m3
  grid (rms 0.83 vs 0.94 at alpha=1).  The DC part (63) is exact via a
  rank-1 correction: out += 63*rowsum(x)[t]*scale[o], folded into PSUM
  eviction at zero extra cost: out = (ps + 63*alpha*rowsum(x)) * (scale/alpha).
- x is split into x_hi = e4m3(x) and x_lo = e4m3(x - x_hi).  For the first
  LO_BLOCKS (16) of the 32 k-blocks, the DoubleRow plane pair carries
  (x_hi[k], x_lo[k]) against a duplicated weight plane -- high x precision.
  For the remaining 16 k-blocks the pair carries (x_hi[k0], x_hi[k1]) against
  (w8[k0], w8[k1]) -- two k-blocks per instruction.
  => 16 + 8 = 24 DoubleRow matmuls per [128,512] PSUM tile.

Error budget (verified bit-faithful against numpy sim on the real inputs):
rel err ~1.77e-2 vs the 2e-2 gate (x-rounding coupling on the k-doubled half
~1.12e-2, fp8 weight quantization ~1.27e-2).
"""

import numpy as np
import ml_dtypes

import concourse.bass as bass
import concourse.mybir as mybir
import concourse.tile as tile
from concourse import bacc
from concourse.bass_utils import run_bass_kernel_spmd

B, S, IN, OUT = 4, 2048, 4096, 11008
N_CORES = 8
TOK = B * S
O_CORE = OUT // N_CORES
P = 128

M_TILE = 256
N_TILE = 512
X_BUFS = 4
PSUM_BUFS = 6
KG = 8          # k-planes per startup weight-DMA chunk
LO_BLOCKS = 16  # k-blocks (of 32) whose DoubleRow pair carries (x_hi, x_lo)
ALPHA = 1.0125  # global fp8 weight-encoding scale
W_BCAST = 1     # 1: lo-group w planes via stride-0 AP; 0: duplicate in DRAM

F8 = ml_dtypes.float8_e4m3  # trn2 float8e4 (e4m3, max 240)


def build_nc(tok=TOK, k_dim=IN, o_core=O_CORE,
             m_tile=M_TILE, n_tile=N_TILE, x_bufs=X_BUFS, psum_bufs=PSUM_BUFS,
             lo_blocks=LO_BLOCKS, w_bcast=W_BCAST):
    assert k_dim % P == 0 and tok % m_tile == 0 and m_tile % P == 0
    ksub = k_dim // P                      # real k-blocks (32)
    assert lo_blocks % 2 == 0 and (ksub - lo_blocks) % 2 == 0
    planes = 2 * lo_blocks + (ksub - lo_blocks)   # x plane count (48)
    w_planes = ksub if w_bcast else planes        # w plane count (32 or 48)
    n_mm = planes // 2                     # DoubleRow matmuls per psum tile (24)
    msub = m_tile // P
    n_mtiles = tok // m_tile
    n_slices = [(n0, min(n_tile, o_core - n0)) for n0 in range(0, o_core, n_tile)]
    f8 = mybir.dt.float8e4

    # w chunk offsets into the per-partition packed layout (see _shard_inputs)
    w_offs = {}
    off = 0
    for n_idx, (n0, nsz) in enumerate(n_slices):
        for kg in range(0, w_planes, KG):
            kgn = min(KG, w_planes - kg)
            w_offs[(n_idx, kg // KG)] = (off, kgn, nsz)
            off += kgn * nsz
    w_total = off

    nc = bacc.Bacc(None, target_bir_lowering=False)
    # xt is pre-tiled host-side: per m-tile, each partition's 48 plane
    # fragments are contiguous (12 KB lines -> full-rate DMA).
    xt = nc.declare_dram_parameter("xt", [n_mtiles * P, planes * m_tile], f8,
                                   False)
    # wt is packed per partition in chunk-consumption order (4 KB lines).
    wt = nc.declare_dram_parameter("wt", [P, w_total], f8, False)
    rs63a = nc.declare_dram_parameter("rs63a", [tok], mybir.dt.float32, False)
    sinva = nc.declare_dram_parameter("sinva", [o_core], mybir.dt.float32, False)
    bias = nc.declare_dram_parameter("bias", [o_core], mybir.dt.float32, False)
    out = nc.declare_dram_parameter("out", [tok, o_core], mybir.dt.float32, True)

    with tile.TileContext(nc) as tc:
        with (
            tc.tile_pool(name="const", bufs=1) as cpool,
            tc.tile_pool(name="xp", bufs=x_bufs) as xpool,
            tc.tile_pool(name="op", bufs=2) as opool,
            tc.tile_pool(name="ps", bufs=psum_bufs, space="PSUM") as pspool,
        ):
            xt_re = xt.rearrange("(m p) f -> m p f", p=P)
            out_re = out.rearrange("(m p) o -> m p o", p=P)
            rs_re = rs63a.rearrange("(m ms p) -> p m ms", ms=msub, p=P)

            # x tiles stream on the sync queue; resident weights load on the
            # scalar queue so they don't serialize behind/ahead of x; the
            # scale/bias broadcasts queue behind the weights so the first
            # x tile gets the bandwidth.
            x_pre = xpool.tile([P, planes, m_tile], f8, tag="x")
            nc.sync.dma_start(out=x_pre[:], in_=xt_re[0])

            w_tiles = {}
            for n_idx, (n0, nsz) in enumerate(n_slices):
                for kg in range(0, w_planes, KG):
                    kgi = kg // KG
                    off, kgn, _ = w_offs[(n_idx, kgi)]
                    w_t = cpool.tile([P, kgn, nsz], f8, tag=f"w_{n_idx}_{kg}")
                    # first two chunks ride the otherwise-idle x queue so the
                    # first psum tile isn't gated on the single scalar queue
                    eng = nc.sync if (n_idx == 0 and kgi < 2) else nc.scalar
                    eng.dma_start(out=w_t[:], in_=wt[:, off:off + kgn * nsz])
                    w_tiles[(n_idx, kgi)] = w_t
            sinva_sb = cpool.tile([P, o_core], mybir.dt.float32)
            nc.scalar.dma_start(out=sinva_sb[:],
                                in_=sinva[None, :].to_broadcast((P, o_core)))
            bias_sb = cpool.tile([P, o_core], mybir.dt.float32)
            nc.scalar.dma_start(out=bias_sb[:],
                                in_=bias[None, :].to_broadcast((P, o_core)))
            rs_sb = cpool.tile([P, n_mtiles, msub], mybir.dt.float32)
            nc.scalar.dma_start(out=rs_sb[:], in_=rs_re[:, :, :])

            for mi in range(n_mtiles):
                if mi == 0:
                    x_sb = x_pre
                else:
                    x_sb = xpool.tile([P, planes, m_tile], f8, tag="x")
                    nc.sync.dma_start(out=x_sb[:], in_=xt_re[mi])
                out_sb = opool.tile([P, msub, o_core], mybir.dt.float32, tag="o")
                for ms in range(msub):
                    for n_idx, (n0, nsz) in enumerate(n_slices):
                        ps = pspool.tile([P, n_tile], mybir.dt.float32, tag="ps")
                        psv = ps[:, :nsz]
                        for t in range(n_mm):
                            if w_bcast:
                                wk = t if t < lo_blocks else 2 * t - lo_blocks
                            else:
                                wk = 2 * t
                            w_t = w_tiles[(n_idx, wk // KG)]
                            ko = wk % KG
                            if w_bcast and t < lo_blocks:
                                rhs = w_t[:, ko:ko + 1, :nsz].to_broadcast(
                                    (P, 2, nsz))
                            else:
                                rhs = w_t[:, ko:ko + 2, :nsz]
                            nc.tensor.matmul(
                                psv,
                                lhsT=x_sb[:, 2 * t:2 * t + 2, ms * P:(ms + 1) * P],
                                rhs=rhs,
                                start=(t == 0),
                                stop=(t == n_mm - 1),
                                perf_mode=mybir.MatmulPerfMode.DoubleRow,
                            )
                        # out = (psum + 63*alpha*rowsum(x)[t]) * (scale/alpha)[o]
                        nc.vector.scalar_tensor_tensor(
                            out=out_sb[:, ms, n0:n0 + nsz],
                            in0=psv,
                            scalar=rs_sb[:, mi, ms:ms + 1],
                            op0=mybir.AluOpType.add,
                            op1=mybir.AluOpType.mult,
                            in1=sinva_sb[:, n0:n0 + nsz],
                        )
                        nc.vector.tensor_add(out=out_sb[:, ms, n0:n0 + nsz],
                                             in0=out_sb[:, ms, n0:n0 + nsz],
                                             in1=bias_sb[:, n0:n0 + nsz])
                        nc.sync.dma_start(
                            out=out_re[mi * msub + ms][:, n0:n0 + nsz],
                            in_=out_sb[:, ms, n0:n0 + nsz])
    nc.compile()
    return nc


def _shard_inputs(x2d, w, scale, bias, n_cores=N_CORES, o_core=O_CORE,
                  lo_blocks=LO_BLOCKS, w_bcast=W_BCAST,
                  m_tile=M_TILE, n_tile=N_TILE):
    ksub = IN // P
    x_hi = x2d.astype(F8)                                 # [TOK, IN] e4m3
    e = x2d - x_hi.astype(np.float32)
    x_lo = e.astype(F8)
    rs63a = np.ascontiguousarray(
        63.0 * ALPHA * x2d.sum(axis=1, dtype=np.float64)).astype(np.float32)

    # x plane order: [x_hi[k], x_lo[k]] pairs for k < lo_blocks, then
    # x_hi[lo_blocks:].  Pre-tiled so each m-tile is one contiguous
    # [planes*m_tile] line per partition: [mi, p, plane, t'].
    xhiT = np.ascontiguousarray(x_hi.T).reshape(ksub, P, TOK)
    xloT = np.ascontiguousarray(x_lo.T).reshape(ksub, P, TOK)
    x_planes = []
    for k in range(lo_blocks):
        x_planes += [xhiT[k], xloT[k]]
    for k in range(lo_blocks, ksub):
        x_planes.append(xhiT[k])
    planes = len(x_planes)
    n_mtiles = TOK // m_tile
    arr = np.stack(x_planes, axis=0)                      # [planes, P, TOK]
    arr = arr.reshape(planes, P, n_mtiles, m_tile)
    xt = np.ascontiguousarray(arr.transpose(2, 1, 0, 3)   # [mi, p, plane, t']
                              ).reshape(n_mtiles * P, planes * m_tile)

    common = {"xt": xt, "rs63a": rs63a}
    in_maps = []
    for c in range(n_cores):
        sl = slice(c * o_core, (c + 1) * o_core)
        w_c = w[sl].astype(np.float32) - 63.0             # [o_core, IN]
        w8 = (w_c * ALPHA).astype(F8)                     # encoded fp8 weights
        w8T = np.ascontiguousarray(w8.T).reshape(ksub, P, o_core)
        if w_bcast:
            wp = w8T                                      # [32, P, o_core]
        else:
            w_planes = []
            for k in range(lo_blocks):
                w_planes += [w8T[k], w8T[k]]
            for k in range(lo_blocks, ksub):
                w_planes.append(w8T[k])
            wp = np.stack(w_planes, axis=0)
        # pack per partition in chunk-consumption order: per (n-slice,
        # k-group) a contiguous [kgn*nsz] block (4 KB DMA lines)
        blocks = []
        for n0 in range(0, o_core, n_tile):
            nsz = min(n_tile, o_core - n0)
            for kg in range(0, wp.shape[0], KG):
                kgn = min(KG, wp.shape[0] - kg)
                blk = wp[kg:kg + kgn, :, n0:n0 + nsz]     # [kgn, P, nsz]
                blocks.append(blk.transpose(1, 0, 2).reshape(P, kgn * nsz))
        wt = np.ascontiguousarray(np.concatenate(blocks, axis=1))  # [P, total]
        in_maps.append({
            **common,
            "wt": wt,
            "sinva": np.ascontiguousarray(scale[sl] / ALPHA),
            "bias": np.ascontiguousarray(bias[sl]),
        })
    return in_maps


def _ensure_ntff_hook():
    """Register the axon NTFF profiling hook if the image's antenv lacks it."""
    import sys, types
    try:
        from antenv.axon_hooks import get_axon_ntff_profile_hook  # noqa: F401
        return
    except ImportError:
        pass
    try:
        import antenv
        from trn_agent_boot.trn_boot import _ntff_profile_via_ctypes
        mod = types.ModuleType("antenv.axon_hooks")
        _hook = [_ntff_profile_via_ctypes("/opt/axon/libaxon_pjrt.so")]
        mod.set_axon_ntff_profile_hook = lambda h: _hook.__setitem__(0, h)
        mod.get_axon_ntff_profile_hook = lambda: _hook[0]
        sys.modules["antenv.axon_hooks"] = mod
        antenv.axon_hooks = mod
    except Exception as e:  # profiling is best-effort; execution still works
        print(f"NTFF hook registration failed: {e}")


def run_hw(x2d, w, scale, bias, trace=False, **build_kwargs):
    """Run sharded on 8 cores; returns (full [TOK, OUT] f32 output, exec_time_ns)."""
    if trace:
        _ensure_ntff_hook()
    nc = build_nc(**build_kwargs)
    in_maps = _shard_inputs(x2d, w, scale, bias,
                            lo_blocks=build_kwargs.get("lo_blocks", LO_BLOCKS),
                            w_bcast=build_kwargs.get("w_bcast", W_BCAST),
                            m_tile=build_kwargs.get("m_tile", M_TILE),
                            n_tile=build_kwargs.get("n_tile", N_TILE))
    last_err = None
    for attempt in range(3):
        try:
            res = run_bass_kernel_spmd(nc, in_maps, core_ids=list(range(N_CORES)),
                                       trace=trace)
            out = np.concatenate([res.results[c]["out"] for c in range(N_CORES)],
                                 axis=1)
            return out, res.exec_time_ns
        except Exception as e:  # transient NRT_EXEC_UNIT_UNRECOVERABLE etc.
            last_err = e
            print(f"run attempt {attempt} failed: {type(e).__name__}: {e}")
            try:
                import jax
                import jax.extend.backend as _jb
                jax.clear_caches()
                _jb.clear_backends()
            except Exception as e2:
                print(f"backend reset failed: {e2}")
            import time
            time.sleep(5)
    raise last_err


def kernel(**inputs):
    x = np.asarray(inputs["x"], dtype=np.float32)
    w = np.asarray(inputs["weight_int8"]).astype(np.int32)
    scale = np.asarray(inputs["scale"], dtype=np.float32)
    bias = np.asarray(inputs["bias"], dtype=np.float32)
    out2d, _ = run_hw(x.reshape(TOK, IN), w, scale, bias, trace=False)
    return out2d.reshape(B, S, OUT)
